# revision 1
# baseline (speedup 1.0000x reference)
"""Trainium2 Bass kernel for nn_ContrastiveDist (supervised contrastive loss).

Math
----
The reference builds (n,n) distance/weight matrices, but the loss collapses
exactly to per-class statistics.  With classes c = 0..15, per-class count
cnt[c], feature sums C[c,:], squared-norm sums SqSum[c], global sums
Ftot / SSall:

    alpha[c] = 1/(cnt[c]-1+eps)
    beta[c]  = 1/(n-cnt[c]+eps)
    loss_i   = sq_i*P[c_i] + (Q[c_i]+M) + f_i . R[c_i]
      P[c]   = alpha*cnt - beta*(n-cnt)
      Q[c]   = alpha*SqSum[c] - beta*(SSall-SqSum[c])
      R[c,:] = 2*beta*(Ftot-C[c]) - 2*alpha*C[c]
    result   = sum(relu(loss_i)*valid_i) / max(sum(valid_i), 1)

valid_i = (cnt[c_i] >= 2) is folded into the coefficients: Raug rows of
invalid classes are zeroed, so relu(loss) = 0 there, and the valid count
comes from sum(cnt[c]*vmask[c]).  Validated to ~4e-7 rel vs the f32
reference (sim).

Distribution: every core redundantly computes the full loss (inputs are
replicated).  No collectives: a cross-core AllGather costs ~9us plus a
~45us rank-skew barrier under this dispatch path, far more than the
~15us of redundant per-core compute it would save.
"""

import numpy as np
import ml_dtypes

import concourse.bacc as bacc
import concourse.tile as tile
import concourse.mybir as mybir
from concourse.bass_utils import run_bass_kernel_spmd

N, D, K, NCORES = 8192, 128, 16, 8
T = N // 128               # 64 row-tiles of 128
W = D + 3                  # faug stride: [F(128), sq, 1, pad]
EPS, MARGIN = 1e-6, 10.0
F32 = mybir.dt.float32
BF16 = mybir.dt.bfloat16
Alu = mybir.AluOpType
Act = mybir.ActivationFunctionType
AxX = mybir.AxisListType.X

# const tensor (128, CW) f32:
#   cols 0:16   iota c (one-hot compare operand, broadcast over tiles)
#   col 16      1.0  (ones(128,1) lhsT for the final partition reduce)
#   cols 17:33  1.0  (rows 0:16 = ones(16,16) lhsT for global-sum broadcast)
CW = 34

_CACHE: dict = {}


def _build():
    if "nc" in _CACHE:
        return _CACHE["nc"]

    nc = bacc.Bacc("TRN2", target_bir_lowering=False, debug=False, num_devices=NCORES)
    fain = nc.dram_tensor("fain", [128, T * W], F32, kind="ExternalInput").ap()
    fhin = nc.dram_tensor("fhin", [128, T * W], BF16, kind="ExternalInput").ap()
    flin = nc.dram_tensor("flin", [128, T * W], BF16, kind="ExternalInput").ap()
    labrep = nc.dram_tensor("labrep", [128, T * 16], F32, kind="ExternalInput").ap()
    lab16 = nc.dram_tensor("lab16", [16, N], BF16, kind="ExternalInput").ap()
    c16b = nc.dram_tensor("c16b", [16, 1], F32, kind="ExternalInput").ap()
    cst = nc.dram_tensor("cst", [128, CW], F32, kind="ExternalInput").ap()
    res = nc.dram_tensor("res", [1, 1], F32, kind="ExternalOutput").ap()

    with tile.TileContext(nc) as tc:
        with (
            tc.tile_pool(name="sb", bufs=1) as sb,
            tc.tile_pool(name="ps", bufs=1, space="PSUM") as ps,
        ):
            # ---------------- loads ----------------
            csts = sb.tile([128, CW], F32)
            nc.sync.dma_start(csts[:], cst)
            labs = sb.tile([128, T * 16], F32)
            nc.gpsimd.dma_start(labs[:], labrep)
            lab16s = sb.tile([16, N], BF16)
            nc.gpsimd.dma_start(lab16s[:], lab16)
            c16s = sb.tile([16, 1], F32)
            nc.gpsimd.dma_start(c16s[:], c16b)

            faug = sb.tile([128, T * W], F32)
            fa3 = faug.rearrange("p (t w) -> p t w", w=W)
            faugh = sb.tile([128, W * T], BF16)
            fh3 = faugh.rearrange("p (w t) -> p w t", t=T)
            faugl = sb.tile([128, W * T], BF16)
            fl3 = faugl.rearrange("p (w t) -> p w t", t=T)
            CH = T * W // 4
            for g in range(4):  # contiguous 2D chunks, alternate queues
                eng = nc.sync if g % 2 == 0 else nc.gpsimd
                eng.dma_start(faug[:, g * CH:(g + 1) * CH],
                              fain[:, g * CH:(g + 1) * CH])
            for g in range(2):
                nc.sync.dma_start(faugh[:, g * 2 * CH:(g + 1) * 2 * CH],
                                  fhin[:, g * 2 * CH:(g + 1) * 2 * CH])
                nc.gpsimd.dma_start(faugl[:, g * 2 * CH:(g + 1) * 2 * CH],
                                    flin[:, g * 2 * CH:(g + 1) * 2 * CH])

            # ---------------- one-hots ----------------
            eohaf = sb.tile([128, T * 16], F32)
            eohf3 = eohaf.rearrange("p (t c) -> p t c", c=16)
            iota3 = csts[:, 0:16].unsqueeze(1).broadcast_to((128, T, 16))
            lab3 = labs.rearrange("p (t c) -> p t c", c=16)
            nc.vector.tensor_tensor(eohf3[:, :, :], iota3, lab3, op=Alu.is_equal)
            eoha = sb.tile([128, T * 16], BF16)
            eoh3 = eoha.rearrange("p (t c) -> p t c", c=16)
            nc.vector.tensor_copy(eoha[:], eohaf[:])
            eohT = sb.tile([16, N], BF16)
            nc.vector.tensor_scalar(eohT[:], lab16s[:], c16s[:], None,
                                    op0=Alu.is_equal)

            # ---------------- sq_i then per-class stats ----------------
            ffbig = sb.tile([128, T * D], F32)
            ff3 = ffbig.rearrange("p (t d) -> p t d", d=D)
            nc.scalar.activation(ff3[:, :, :], fa3[:, :, 0:D], Act.Square)
            sqd = sb.tile([128, T], F32)
            nc.vector.tensor_reduce(sqd[:], ff3, axis=AxX, op=Alu.add)
            nc.vector.tensor_copy(fa3[:, :, D], sqd[:])
            nc.vector.tensor_copy(fh3[:, D, :], sqd[:])
            nc.vector.scalar_tensor_tensor(fl3[:, D, :], sqd[:], 0.0,
                                           fh3[:, D, :],
                                           op0=Alu.bypass, op1=Alu.subtract)

            statsP = ps.tile([16, D + 2], F32)
            for t in range(T):
                nc.tensor.matmul(statsP[:], eoh3[:, t, :], fh3[:, 0:D + 2, t],
                                 start=(t == 0), stop=False)
                nc.tensor.matmul(statsP[:], eoh3[:, t, :], fl3[:, 0:D + 2, t],
                                 start=False, stop=(t == T - 1))
            stats = sb.tile([16, D + 2], F32)
            nc.vector.tensor_copy(stats[:], statsP[:])

            # ---------------- per-class coefficients ----------------
            C = stats[:, 0:D]
            SqS = stats[:, D:D + 1]
            cnt = stats[:, D + 1:D + 2]
            gbP = ps.tile([16, D + 2], F32)
            nc.tensor.matmul(gbP[:], csts[0:16, 17:33], stats[:],
                             start=True, stop=True)
            gb = sb.tile([16, D + 2], F32)
            nc.vector.tensor_copy(gb[:], gbP[:])
            Ftot = gb[:, 0:D]
            SSall = gb[:, D:D + 1]

            alpha = sb.tile([16, 1], F32)
            nc.vector.tensor_scalar(alpha[:], cnt, EPS - 1.0, None, op0=Alu.add)
            nc.vector.reciprocal(alpha[:], alpha[:])
            beta = sb.tile([16, 1], F32)
            nc.vector.tensor_scalar(beta[:], cnt, -1.0, float(N) + EPS,
                                    op0=Alu.mult, op1=Alu.add)
            nc.vector.reciprocal(beta[:], beta[:])
            nalpha2 = sb.tile([16, 1], F32)
            nc.vector.tensor_scalar(nalpha2[:], alpha[:], -2.0, None, op0=Alu.mult)
            beta2 = sb.tile([16, 1], F32)
            nc.vector.tensor_scalar(beta2[:], beta[:], 2.0, None, op0=Alu.mult)

            raug = sb.tile([16, D + 2], F32)
            tmpd = sb.tile([16, D], F32)
            nc.vector.tensor_tensor(tmpd[:], Ftot, C, op=Alu.subtract)
            nc.vector.tensor_scalar(tmpd[:], tmpd[:], beta2[:], None, op0=Alu.mult)
            nc.vector.scalar_tensor_tensor(raug[:, 0:D], C, nalpha2[:], tmpd[:],
                                           op0=Alu.mult, op1=Alu.add)
            nmc = sb.tile([16, 1], F32)
            nc.vector.tensor_scalar(nmc[:], cnt, -1.0, float(N),
                                    op0=Alu.mult, op1=Alu.add)
            nc.vector.tensor_tensor(nmc[:], nmc[:], beta[:], op=Alu.mult)
            nc.vector.scalar_tensor_tensor(raug[:, D:D + 1], cnt, alpha[:], nmc[:],
                                           op0=Alu.mult, op1=Alu.subtract)
            ssd = sb.tile([16, 1], F32)
            nc.vector.tensor_tensor(ssd[:], SSall, SqS, op=Alu.subtract)
            nc.vector.tensor_tensor(ssd[:], ssd[:], beta[:], op=Alu.mult)
            qa = sb.tile([16, 1], F32)
            nc.vector.scalar_tensor_tensor(qa[:], SqS, alpha[:], ssd[:],
                                           op0=Alu.mult, op1=Alu.subtract)
            nc.vector.tensor_scalar(raug[:, D + 1:D + 2], qa[:], MARGIN, None,
                                    op0=Alu.add)

            # fold validity into the coefficients: zero Raug rows of classes
            # with cnt < 2, so relu(loss) vanishes for invalid rows
            vmask = sb.tile([16, 1], F32)
            nc.vector.tensor_scalar(vmask[:], cnt, 1.5, None, op0=Alu.is_ge)
            nc.vector.tensor_scalar(raug[:], raug[:], vmask[:], None, op0=Alu.mult)

            # bf16 hi/lo split of raug -> two-chain bf16 matmul ~= fp32 exact
            rhi = sb.tile([16, D + 2], BF16)
            nc.vector.tensor_copy(rhi[:], raug[:])
            rlo32 = sb.tile([16, D + 2], F32)
            nc.vector.tensor_tensor(rlo32[:], raug[:], rhi[:], op=Alu.subtract)
            rlo = sb.tile([16, D + 2], BF16)
            nc.vector.tensor_copy(rlo[:], rlo32[:])

            # ---------------- per-row losses ----------------
            lossrows = sb.tile([128, T], F32)
            for r in range(T // 2):  # 32 rounds x 2 tiles; D-psum 2 banks x2 slots
                dP = ps.tile([128, 2 * 512], F32, tag="dpsum", bufs=2,
                             name=f"dP{r}")
                d3 = dP.rearrange("p (b x) -> p b x", x=512)
                for j in range(2):
                    t = r * 2 + j
                    lhs = eohT[:, t * 128:(t + 1) * 128]
                    nc.tensor.matmul(d3[:, j, 0:D + 2], lhs, rhi[:],
                                     start=True, stop=False)
                    nc.tensor.matmul(d3[:, j, 0:D + 2], lhs, rlo[:],
                                     start=False, stop=True)
                for j in range(2):
                    t = r * 2 + j
                    pscr = sb.tile([128, D + 2], F32, tag="pscr", bufs=4,
                                   name=f"ps{r}_{j}")
                    nc.vector.scalar_tensor_tensor(
                        pscr[:], d3[:, j, 0:D + 2], 0.0, fa3[:, t, 0:D + 2],
                        op0=Alu.bypass, op1=Alu.mult,
                        accum_out=lossrows[:, t:t + 1])

            # ---------------- final reduction ----------------
            accpair = sb.tile([128, 2], F32)
            nc.gpsimd.memset(accpair[:, 1:2], 0.0)
            relscr = sb.tile([128, T], F32)
            nc.vector.tensor_scalar(relscr[:], lossrows[:], 0.0, None,
                                    op0=Alu.max, op1=Alu.add,
                                    accum_out=accpair[:, 0:1])
            nc.vector.tensor_tensor(accpair[0:16, 1:2], cnt, vmask[:],
                                    op=Alu.mult)
            finP = ps.tile([1, 2], F32)
            nc.tensor.matmul(finP[:], csts[:, 16:17], accpair[:],
                             start=True, stop=True)
            fin = sb.tile([1, 2], F32)
            nc.vector.tensor_copy(fin[:], finP[:])
            den = sb.tile([1, 1], F32)
            nc.vector.tensor_scalar(den[:], fin[:, 1:2], 1.0, None, op0=Alu.max)
            nc.vector.reciprocal(den[:], den[:])
            resS = sb.tile([1, 1], F32)
            nc.vector.tensor_tensor(resS[:], fin[:, 0:1], den[:], op=Alu.mult)
            nc.sync.dma_start(res, resS[:])

    nc.compile()
    _CACHE["nc"] = nc
    return nc


def _make_in_maps(features, labels):
    feats = np.ascontiguousarray(np.asarray(features, dtype=np.float32))
    lab = np.ascontiguousarray(np.asarray(labels)).astype(np.float32)

    cst = np.zeros((128, CW), np.float32)
    cst[:, 0:16] = np.arange(16, dtype=np.float32)[None, :]
    cst[:, 16:33] = 1.0

    fa = np.zeros((128, T, W), np.float32)
    fa[:, :, 0:D] = feats.reshape(T, 128, D).transpose(1, 0, 2)
    fa[:, :, D + 1] = 1.0
    fawt = np.ascontiguousarray(fa.transpose(0, 2, 1))  # (128, W, T)
    fh = fawt.reshape(128, W * T).astype(ml_dtypes.bfloat16)
    fl = (fawt.reshape(128, W * T) - fh.astype(np.float32)).astype(
        ml_dtypes.bfloat16)
    fa = fa.reshape(128, T * W)

    one = {
        "fain": fa,
        "fhin": fh,
        "flin": fl,
        "labrep": np.ascontiguousarray(
            np.repeat(lab.reshape(T, 128).T, 16, axis=1)),
        "lab16": np.ascontiguousarray(
            np.broadcast_to(lab, (16, N))).astype(ml_dtypes.bfloat16),
        "c16b": np.arange(16, dtype=np.float32).reshape(16, 1),
        "cst": cst,
    }
    return [dict(one) for _ in range(NCORES)]


def kernel(features, labels):
    nc = _build()
    in_maps = _make_in_maps(features, labels)
    out = run_bass_kernel_spmd(nc, in_maps, core_ids=list(range(NCORES)))
    return np.float32(out.results[0]["res"][0, 0])



# revision 7
# speedup vs baseline: 2.1818x; 2.1818x over previous
"""Trainium2 Bass kernel for nn_ContrastiveDist (supervised contrastive loss).

Math
----
The (n,n) distance/weight loss collapses to per-class statistics.  With
classes c = 0..15, cnt[c], class feature sums C[c,:], squared-norm sums
SqSum[c], global sums Ftot / SSall:

    alpha[c] = 1/(cnt[c]-1+eps),  beta[c] = 1/(n-cnt[c]+eps)
    P[c]   = alpha*cnt - beta*(n-cnt)
    Q[c]   = alpha*SqSum - beta*(SSall-SqSum)
    R[c,:] = 2*beta*(Ftot-C) - 2*alpha*C
    loss_i = f_i . R[c_i] + sq_i*P[c_i] + Q[c_i] + M
    result = sum(relu(loss_i)*valid_i) / max(sum(valid_i), 1)

Device pipeline (per core, replicated on all 8 cores, no collectives):
  1. DMA features bf16 in two layouts: row-tiled [F|1] (stats) and
     d-major F^T (loss), plus two one-hot layouts.
  2. ACT squares both layouts chunk-wise ([F|1]^2 gives [F^2|1]).
  3. Stats: one PE chain: eoh_t^T @ [F|1|F^2|1] accumulated over the 64
     row tiles -> (16, 258) = [C | cnt | Msq | cnt]; SqSum = rowsum(Msq).
  4. Loss matrix L[c,i] = F^T.R^T + (F^T)^2.P_bcast as 32 FD-512 matmuls
     into a (128, 2048) PSUM laid out as 4 row-groups x 32 classes
     (classes duplicated into both 16-halves to keep PSUM defined).
  5. Mask: t1 = (L + (Q+M)_percls) * onehot_cg; relu; free-axis accum;
     final partition reduce via ones-matmul; divide; DMA scalar out.

valid_i folds into the coefficients (R,P,(Q+M) zeroed for cnt<2).
Validated ~4.5e-5 rel vs fp64 in numpy bf16 simulation.
"""

import numpy as np
import ml_dtypes

import concourse.bacc as bacc
import concourse.tile as tile
import concourse.mybir as mybir
from concourse.bass_utils import run_bass_kernel_spmd

N, D, K, NCORES = 8192, 128, 16, 8
T = N // 128               # 64 row tiles of 128
W = D + 1                  # 129: [F | 1]
NG, GI = 4, 2048           # 4 row-groups of 2048 rows (loss layout)
EPS, MARGIN = 1e-6, 10.0
F32 = mybir.dt.float32
BF16 = mybir.dt.bfloat16
Alu = mybir.AluOpType
Act = mybir.ActivationFunctionType
AxX = mybir.AxisListType.X
BF = ml_dtypes.bfloat16

_CACHE: dict = {}


def _build():
    if "nc" in _CACHE:
        return _CACHE["nc"]

    nc = bacc.Bacc("TRN2", target_bir_lowering=False, debug=False,
                   num_devices=NCORES)
    frd = nc.dram_tensor("frd", [128, T * W], BF16, kind="ExternalInput").ap()
    ftd = nc.dram_tensor("ftd", [128, N], BF16, kind="ExternalInput").ap()
    eohb = nc.dram_tensor("eohb", [128, T * 16], BF16, kind="ExternalInput").ap()
    eohc = nc.dram_tensor("eohc", [128, GI], BF16, kind="ExternalInput").ap()
    csta = nc.dram_tensor("csta", [16, 272], F32, kind="ExternalInput").ap()
    cstb = nc.dram_tensor("cstb", [128, 1], F32, kind="ExternalInput").ap()
    res = nc.dram_tensor("res", [1, 1], F32, kind="ExternalOutput").ap()

    with tile.TileContext(nc) as tc:
        with (
            tc.tile_pool(name="sb", bufs=1) as sb,
            tc.tile_pool(name="ps", bufs=1, space="PSUM") as ps,
        ):
            # ---------------- loads ----------------
            cstas = sb.tile([16, 272], F32)
            nc.sync.dma_start(cstas[:], csta)
            cstbs = sb.tile([128, 1], F32)
            nc.sync.dma_start(cstbs[:], cstb)
            ssel = cstas[:, 0:128]      # Qcg scatter matrix
            idf = cstas[:, 128:144]     # identity 16x16
            ones16 = cstas[:, 144:160]  # ones (16,16)
            onesw = cstas[:, 144:272]   # ones (16,128)

            eohbs = sb.tile([128, T * 16], BF16)
            nc.gpsimd.dma_start(eohbs[:], eohb)

            frgr = sb.tile([128, 2 * T * W], BF16)   # [F|1] rows then [F^2|1]
            fts = sb.tile([128, N], BF16)
            gts = sb.tile([128, N], BF16)
            eohcs = sb.tile([128, GI], BF16)
            NCH = 4
            FCH = T * W // NCH   # 2064
            TCH = N // NCH       # 2048
            for k in range(NCH):
                nc.sync.dma_start(frgr[:, k * FCH:(k + 1) * FCH],
                                  frd[:, k * FCH:(k + 1) * FCH])
                nc.gpsimd.dma_start(fts[:, k * TCH:(k + 1) * TCH],
                                    ftd[:, k * TCH:(k + 1) * TCH])
            nc.gpsimd.dma_start(eohcs[:], eohc)

            # ---------------- squares (ACT), chunk-wise ----------------
            for k in range(NCH):
                nc.scalar.activation(
                    frgr[:, T * W + k * FCH:T * W + (k + 1) * FCH],
                    frgr[:, k * FCH:(k + 1) * FCH], Act.Square)
                nc.scalar.activation(gts[:, k * TCH:(k + 1) * TCH],
                                     fts[:, k * TCH:(k + 1) * TCH], Act.Square)

            # bank-clear weights for the loss PSUM (zeros)
            zb = sb.tile([128, 128], BF16)
            nc.gpsimd.memset(zb[:], 0.0)

            # ---------------- stats: eoh^T @ [F|1|F^2|1] ----------------
            # single accumulation chain; rhs is a 2-segment AP covering the
            # [F|1] tile and its squared twin in one FD=258 matmul
            frgr4 = frgr.rearrange("p (s t w) -> p t s w", s=2, w=W)
            eohb3 = eohbs.rearrange("p (t c) -> p t c", c=16)
            statsP = ps.tile([16, 2 * W], F32)
            for t in range(T):
                nc.tensor.matmul(statsP[:], eohb3[:, t, :], frgr4[:, t, :, :],
                                 start=(t == 0), stop=(t == T - 1))

            # ---------------- coefficients: P path first ----------------
            cntS = sb.tile([16, 1], F32)
            nc.vector.tensor_copy(cntS[:], statsP[:, D:D + 1])
            alpha = sb.tile([16, 1], F32)
            nc.vector.tensor_scalar(alpha[:], cntS[:], EPS - 1.0, None,
                                    op0=Alu.add)
            nc.vector.reciprocal(alpha[:], alpha[:])
            beta = sb.tile([16, 1], F32)
            nc.vector.tensor_scalar(beta[:], cntS[:], -1.0, float(N) + EPS,
                                    op0=Alu.mult, op1=Alu.add)
            nc.vector.reciprocal(beta[:], beta[:])
            vmask = sb.tile([16, 1], F32)
            nc.vector.tensor_scalar(vmask[:], cntS[:], 1.5, None, op0=Alu.is_ge)
            nmc = sb.tile([16, 1], F32)
            nc.vector.tensor_scalar(nmc[:], cntS[:], -1.0, float(N),
                                    op0=Alu.mult, op1=Alu.add)
            nc.vector.tensor_tensor(nmc[:], nmc[:], beta[:], op=Alu.mult)
            Pv = sb.tile([16, 1], F32)
            nc.vector.scalar_tensor_tensor(Pv[:], cntS[:], alpha[:], nmc[:],
                                           op0=Alu.mult, op1=Alu.subtract)
            nc.vector.tensor_tensor(Pv[:], Pv[:], vmask[:], op=Alu.mult)
            Pstack = sb.tile([16, 128], F32)
            nc.vector.tensor_scalar(Pstack[:], onesw, Pv[:], None, op0=Alu.mult)
            pcoef = ps.tile([128, 33], F32)
            nc.tensor.transpose(pcoef[:, 16:32], Pstack[:], idf)
            Pbb = sb.tile([128, 32], BF16)
            nc.vector.tensor_copy(Pbb[:, 0:16], pcoef[:, 16:32])
            nc.vector.tensor_copy(Pbb[:, 16:32], pcoef[:, 16:32])

            # ---------------- loss chain B: sq_i * P[c]  (g-major) -------
            # PSUM bank discipline: one full-partition zero matmul per bank
            # clears the has_written bits (start=True clears the WHOLE bank);
            # every real matmul then uses start=False, which overwrites where
            # the bit is clear and accumulates where it is set.  The dummy
            # overlaps every later region, so Tile orders it first.
            lossP = ps.tile([128, GI], F32)
            for h in range(4):
                nc.tensor.matmul(lossP[:, 512 * h:512 * h + 16], zb[:],
                                 fts[:, 0:16], start=True, stop=False,
                                 skip_group_check=True)
            for g in range(NG):
                for h in range(4):
                    nc.tensor.matmul(
                        lossP[32 * g:32 * g + 32, 512 * h:512 * (h + 1)],
                        Pbb[:], gts[:, GI * g + 512 * h:GI * g + 512 * (h + 1)],
                        start=False, stop=False, tile_position=(0, 32 * g),
                        skip_group_check=True)

            # ---------------- coefficients: R and Q paths ----------------
            statsS = sb.tile([16, 2 * W], F32)
            nc.vector.tensor_copy(statsS[:], statsP[:])
            SqS = sb.tile([16, 1], F32)
            nc.vector.tensor_reduce(SqS[:], statsS[:, W:W + D], axis=AxX,
                                    op=Alu.add)
            gbP = ps.tile([16, D + 1], F32)
            nc.tensor.matmul(gbP[:, 0:D], ones16, statsS[:, 0:D],
                             start=True, stop=True)
            nc.tensor.matmul(gbP[:, D:D + 1], ones16, SqS[:],
                             start=True, stop=True)
            tmpd = sb.tile([16, D], F32)
            nc.vector.tensor_tensor(tmpd[:], gbP[:, 0:D], statsS[:, 0:D],
                                    op=Alu.subtract)
            beta2 = sb.tile([16, 1], F32)
            nc.vector.tensor_scalar(beta2[:], beta[:], 2.0, None, op0=Alu.mult)
            nc.vector.tensor_scalar(tmpd[:], tmpd[:], beta2[:], None,
                                    op0=Alu.mult)
            nalpha2 = sb.tile([16, 1], F32)
            nc.vector.tensor_scalar(nalpha2[:], alpha[:], -2.0, None,
                                    op0=Alu.mult)
            Rv = sb.tile([16, D], F32)
            nc.vector.scalar_tensor_tensor(Rv[:], statsS[:, 0:D], nalpha2[:],
                                           tmpd[:], op0=Alu.mult, op1=Alu.add)
            nc.vector.tensor_scalar(Rv[:], Rv[:], vmask[:], None, op0=Alu.mult)
            nc.tensor.transpose(pcoef[:, 0:16], Rv[:], idf)
            RTb = sb.tile([128, 32], BF16)
            nc.vector.tensor_copy(RTb[:, 0:16], pcoef[:, 0:16])
            nc.vector.tensor_copy(RTb[:, 16:32], pcoef[:, 0:16])

            ssd = sb.tile([16, 1], F32)
            nc.vector.tensor_tensor(ssd[:], gbP[:, D:D + 1], SqS[:],
                                    op=Alu.subtract)
            nc.vector.tensor_tensor(ssd[:], ssd[:], beta[:], op=Alu.mult)
            QM = sb.tile([16, 1], F32)
            nc.vector.scalar_tensor_tensor(QM[:], SqS[:], alpha[:], ssd[:],
                                           op0=Alu.mult, op1=Alu.subtract)
            nc.vector.tensor_scalar(QM[:], QM[:], MARGIN, vmask[:],
                                    op0=Alu.add, op1=Alu.mult)
            nc.tensor.matmul(pcoef[:, 32:33], ssel, QM[:], start=True,
                             stop=True)
            Qcg = sb.tile([128, 1], F32)
            nc.vector.tensor_copy(Qcg[:], pcoef[:, 32:33])

            # ---------------- loss chain A: f_i . R[c]  (h-major) --------
            for h in range(4):
                for g in range(NG):
                    nc.tensor.matmul(
                        lossP[32 * g:32 * g + 32, 512 * h:512 * (h + 1)],
                        RTb[:], fts[:, GI * g + 512 * h:GI * g + 512 * (h + 1)],
                        start=False, stop=(g == NG - 1),
                        tile_position=(0, 32 * g), skip_group_check=True)

            # ---------------- mask + relu + accumulate -------------------
            accq = sb.tile([128, 8], F32)
            nc.gpsimd.memset(accq[:], 0.0)
            nc.vector.tensor_tensor(accq[0:16, 4:5], cntS[:], vmask[:],
                                    op=Alu.mult)
            for h in range(4):
                t1 = sb.tile([128, 512], F32, tag="t1", bufs=2, name=f"t1_{h}")
                nc.vector.scalar_tensor_tensor(
                    t1[:], lossP[:, 512 * h:512 * (h + 1)], Qcg[:],
                    eohcs[:, 512 * h:512 * (h + 1)],
                    op0=Alu.add, op1=Alu.mult)
                rel = sb.tile([128, 512], F32, tag="rel", bufs=2,
                              name=f"rel_{h}")
                nc.vector.tensor_scalar(rel[:], t1[:], 0.0, None,
                                        op0=Alu.max, op1=Alu.add,
                                        accum_out=accq[:, h:h + 1])

            # ---------------- final reduction ----------------
            finP = ps.tile([1, 8], F32)
            nc.tensor.matmul(finP[:], cstbs[:], accq[:], start=True, stop=True)
            fin = sb.tile([1, 8], F32)
            nc.vector.tensor_copy(fin[:], finP[:])
            numS = sb.tile([1, 1], F32)
            nc.vector.tensor_reduce(numS[:], fin[:, 0:4], axis=AxX, op=Alu.add)
            den = sb.tile([1, 1], F32)
            nc.vector.tensor_scalar(den[:], fin[:, 4:5], 1.0, None, op0=Alu.max)
            nc.vector.reciprocal(den[:], den[:])
            resS = sb.tile([1, 1], F32)
            nc.vector.tensor_tensor(resS[:], numS[:], den[:], op=Alu.mult)
            nc.sync.dma_start(res, resS[:])

    nc.compile()
    _CACHE["nc"] = nc
    return nc


def _make_in_maps(features, labels):
    feats = np.ascontiguousarray(np.asarray(features, dtype=np.float32))
    lab = np.ascontiguousarray(np.asarray(labels)).astype(np.int64)
    fb = feats.astype(BF)                                   # (8192, 128)

    fr = np.zeros((128, T, W), BF)
    fr[:, :, 0:D] = fb.reshape(T, 128, D).transpose(1, 0, 2)
    fr[:, :, D] = 1.0
    ft = np.ascontiguousarray(fb.T)                         # (128, 8192)

    eoh = (lab[:, None] == np.arange(K)[None, :])           # (8192, 16)
    eohb = np.ascontiguousarray(
        eoh.reshape(T, 128, K).transpose(1, 0, 2).reshape(128, T * K)
    ).astype(BF)
    eohcg = np.zeros((128, GI), BF)
    for g in range(NG):
        eohcg[32 * g:32 * g + K, :] = eoh[g * GI:(g + 1) * GI, :].T

    csta = np.zeros((16, 272), np.float32)
    for g in range(NG):
        for q in range(K):
            csta[q, 32 * g + q] = 1.0
            csta[q, 32 * g + 16 + q] = 1.0
    csta[:, 128:144] = np.eye(16, dtype=np.float32)
    csta[:, 144:272] = 1.0
    cstb = np.ones((128, 1), np.float32)

    one = {
        "frd": np.ascontiguousarray(fr.reshape(128, T * W)),
        "ftd": ft,
        "eohb": eohb,
        "eohc": eohcg,
        "csta": csta,
        "cstb": cstb,
    }
    return [dict(one) for _ in range(NCORES)]


def kernel(features, labels):
    nc = _build()
    in_maps = _make_in_maps(features, labels)
    out = run_bass_kernel_spmd(nc, in_maps, core_ids=list(range(NCORES)))
    return np.float32(out.results[0]["res"][0, 0])


# revision 8
# speedup vs baseline: 2.2591x; 1.0354x over previous
"""Trainium2 Bass kernel for nn_ContrastiveDist (supervised contrastive loss).

Math
----
The (n,n) distance/weight loss collapses to per-class statistics.  With
classes c = 0..15, cnt[c], class feature sums C[c,:], squared-norm sums
SqSum[c], global sums Ftot / SSall:

    alpha[c] = 1/(cnt[c]-1+eps),  beta[c] = 1/(n-cnt[c]+eps)
    P[c]   = alpha*cnt - beta*(n-cnt)
    Q[c]   = alpha*SqSum - beta*(SSall-SqSum)
    R[c,:] = 2*beta*(Ftot-C) - 2*alpha*C
    loss_i = f_i . R[c_i] + sq_i*P[c_i] + Q[c_i] + M
    result = sum(relu(loss_i)*valid_i) / max(sum(valid_i), 1)

Device pipeline (per core, replicated on all 8 cores, no collectives):
  1. DMA features bf16 in two layouts: row-tiled [F|1] (stats) and
     d-major F^T (loss), plus two one-hot layouts.
  2. ACT squares both layouts chunk-wise ([F|1]^2 gives [F^2|1]).
  3. Stats: one PE chain: eoh_t^T @ [F|1|F^2|1] accumulated over the 64
     row tiles -> (16, 258) = [C | cnt | Msq | cnt]; SqSum = rowsum(Msq).
  4. Loss matrix L[c,i] = F^T.R^T + (F^T)^2.P_bcast as 32 FD-512 matmuls
     into a (128, 2048) PSUM laid out as 4 row-groups x 32 classes
     (classes duplicated into both 16-halves to keep PSUM defined).
  5. Mask: t1 = (L + (Q+M)_percls) * onehot_cg; relu; free-axis accum;
     final partition reduce via ones-matmul; divide; DMA scalar out.

valid_i folds into the coefficients (R,P,(Q+M) zeroed for cnt<2).
Validated ~4.5e-5 rel vs fp64 in numpy bf16 simulation.
"""

import numpy as np
import ml_dtypes

import concourse.bacc as bacc
import concourse.tile as tile
import concourse.mybir as mybir
from concourse.bass_utils import run_bass_kernel_spmd

N, D, K, NCORES = 8192, 128, 16, 8
T = N // 128               # 64 row tiles of 128
W = D + 1                  # 129: [F | 1]
NG, GI = 4, 2048           # 4 row-groups of 2048 rows (loss layout)
EPS, MARGIN = 1e-6, 10.0
F32 = mybir.dt.float32
BF16 = mybir.dt.bfloat16
Alu = mybir.AluOpType
Act = mybir.ActivationFunctionType
AxX = mybir.AxisListType.X
BF = ml_dtypes.bfloat16

_CACHE: dict = {}


def _build():
    if "nc" in _CACHE:
        return _CACHE["nc"]

    nc = bacc.Bacc("TRN2", target_bir_lowering=False, debug=False,
                   num_devices=NCORES)
    frd = nc.dram_tensor("frd", [128, T * W], BF16, kind="ExternalInput").ap()
    ftd = nc.dram_tensor("ftd", [128, N], BF16, kind="ExternalInput").ap()
    eohb = nc.dram_tensor("eohb", [128, T * 16], BF16, kind="ExternalInput").ap()
    eohc = nc.dram_tensor("eohc", [128, GI], BF16, kind="ExternalInput").ap()
    csta = nc.dram_tensor("csta", [16, 272], F32, kind="ExternalInput").ap()
    cstb = nc.dram_tensor("cstb", [128, 1], F32, kind="ExternalInput").ap()
    res = nc.dram_tensor("res", [1, 1], F32, kind="ExternalOutput").ap()

    with tile.TileContext(nc) as tc:
        with (
            tc.tile_pool(name="sb", bufs=1) as sb,
            tc.tile_pool(name="ps", bufs=1, space="PSUM") as ps,
        ):
            # ---------------- loads ----------------
            cstas = sb.tile([16, 272], F32)
            nc.sync.dma_start(cstas[:], csta)
            cstbs = sb.tile([128, 1], F32)
            nc.sync.dma_start(cstbs[:], cstb)
            ssel = cstas[:, 0:128]      # Qcg scatter matrix
            idf = cstas[:, 128:144]     # identity 16x16
            ones16 = cstas[:, 144:160]  # ones (16,16)
            onesw = cstas[:, 144:272]   # ones (16,128)

            eohbs = sb.tile([128, T * 16], BF16)
            nc.gpsimd.dma_start(eohbs[:], eohb)

            frgr = sb.tile([128, 2 * T * W], BF16)   # [F|1] rows then [F^2|1]
            fts = sb.tile([128, N], BF16)
            gts = sb.tile([128, N], BF16)
            eohcs = sb.tile([128, GI], BF16)
            NCF = 8
            FCH = T * W // NCF   # 1032 (8 row tiles)
            NCH = 4
            TCH = N // NCH       # 2048
            for k in range(NCF):
                nc.sync.dma_start(frgr[:, k * FCH:(k + 1) * FCH],
                                  frd[:, k * FCH:(k + 1) * FCH])
            for k in range(NCH):
                nc.scalar.dma_start(fts[:, k * TCH:(k + 1) * TCH],
                                    ftd[:, k * TCH:(k + 1) * TCH])
            nc.gpsimd.dma_start(eohcs[:], eohc)

            # ------- squares, chunk-wise, split across ACT and DVE -------
            # (ACT has no 16-bit accel for Square: ~2ns/elem; DVE
            # tensor_tensor bf16 runs 2x: ~0.52ns/elem)
            for k in range(NCF):
                src = frgr[:, k * FCH:(k + 1) * FCH]
                dst = frgr[:, T * W + k * FCH:T * W + (k + 1) * FCH]
                if k % 2 == 0:
                    nc.scalar.activation(dst, src, Act.Square)
                else:
                    nc.vector.tensor_tensor(dst, src, src, op=Alu.mult)
            for k in range(NCH):
                src = fts[:, k * TCH:(k + 1) * TCH]
                nc.vector.tensor_tensor(gts[:, k * TCH:(k + 1) * TCH],
                                        src, src, op=Alu.mult)

            # bank-clear weights for the loss PSUM (zeros)
            zb = sb.tile([128, 128], BF16)
            nc.gpsimd.memset(zb[:], 0.0)

            # ---------------- stats: eoh^T @ [F|1|F^2|1] ----------------
            # single accumulation chain; rhs is a 2-segment AP covering the
            # [F|1] tile and its squared twin in one FD=258 matmul
            frgr4 = frgr.rearrange("p (s t w) -> p t s w", s=2, w=W)
            eohb3 = eohbs.rearrange("p (t c) -> p t c", c=16)
            statsP = ps.tile([16, 2 * W], F32)
            for t in range(T):
                nc.tensor.matmul(statsP[:], eohb3[:, t, :], frgr4[:, t, :, :],
                                 start=(t == 0), stop=(t == T - 1))

            # ---------------- coefficients: P path first ----------------
            cntS = sb.tile([16, 1], F32)
            nc.vector.tensor_copy(cntS[:], statsP[:, D:D + 1])
            alpha = sb.tile([16, 1], F32)
            nc.vector.tensor_scalar(alpha[:], cntS[:], EPS - 1.0, None,
                                    op0=Alu.add)
            nc.vector.reciprocal(alpha[:], alpha[:])
            beta = sb.tile([16, 1], F32)
            nc.vector.tensor_scalar(beta[:], cntS[:], -1.0, float(N) + EPS,
                                    op0=Alu.mult, op1=Alu.add)
            nc.vector.reciprocal(beta[:], beta[:])
            vmask = sb.tile([16, 1], F32)
            nc.vector.tensor_scalar(vmask[:], cntS[:], 1.5, None, op0=Alu.is_ge)
            nmc = sb.tile([16, 1], F32)
            nc.vector.tensor_scalar(nmc[:], cntS[:], -1.0, float(N),
                                    op0=Alu.mult, op1=Alu.add)
            nc.vector.tensor_tensor(nmc[:], nmc[:], beta[:], op=Alu.mult)
            Pv = sb.tile([16, 1], F32)
            nc.vector.scalar_tensor_tensor(Pv[:], cntS[:], alpha[:], nmc[:],
                                           op0=Alu.mult, op1=Alu.subtract)
            nc.vector.tensor_tensor(Pv[:], Pv[:], vmask[:], op=Alu.mult)
            Pstack = sb.tile([16, 128], F32)
            nc.vector.tensor_scalar(Pstack[:], onesw, Pv[:], None, op0=Alu.mult)
            pcoef = ps.tile([128, 33], F32)
            nc.tensor.transpose(pcoef[:, 16:32], Pstack[:], idf)
            Pbb = sb.tile([128, 32], BF16)
            nc.vector.tensor_copy(Pbb[:, 0:16], pcoef[:, 16:32])
            nc.vector.tensor_copy(Pbb[:, 16:32], pcoef[:, 16:32])

            # ---------------- loss chain B: sq_i * P[c]  (g-major) -------
            # PSUM bank discipline: one full-partition zero matmul per bank
            # clears the has_written bits (start=True clears the WHOLE bank);
            # every real matmul then uses start=False, which overwrites where
            # the bit is clear and accumulates where it is set.  The dummy
            # overlaps every later region, so Tile orders it first.
            lossP = ps.tile([128, GI], F32)
            for h in range(4):
                nc.tensor.matmul(lossP[:, 512 * h:512 * h + 16], zb[:],
                                 fts[:, 0:16], start=True, stop=False,
                                 skip_group_check=True)
            for g in range(NG):
                for h in range(4):
                    nc.tensor.matmul(
                        lossP[32 * g:32 * g + 32, 512 * h:512 * (h + 1)],
                        Pbb[:], gts[:, GI * g + 512 * h:GI * g + 512 * (h + 1)],
                        start=False, stop=False, tile_position=(0, 32 * g),
                        skip_group_check=True)

            # ---------------- coefficients: R and Q paths ----------------
            statsS = sb.tile([16, 2 * W], F32)
            nc.vector.tensor_copy(statsS[:], statsP[:])
            SqS = sb.tile([16, 1], F32)
            nc.vector.tensor_reduce(SqS[:], statsS[:, W:W + D], axis=AxX,
                                    op=Alu.add)
            gbP = ps.tile([16, D + 1], F32)
            nc.tensor.matmul(gbP[:, 0:D], ones16, statsS[:, 0:D],
                             start=True, stop=True)
            nc.tensor.matmul(gbP[:, D:D + 1], ones16, SqS[:],
                             start=True, stop=True)
            tmpd = sb.tile([16, D], F32)
            nc.vector.tensor_tensor(tmpd[:], gbP[:, 0:D], statsS[:, 0:D],
                                    op=Alu.subtract)
            beta2 = sb.tile([16, 1], F32)
            nc.vector.tensor_scalar(beta2[:], beta[:], 2.0, None, op0=Alu.mult)
            nc.vector.tensor_scalar(tmpd[:], tmpd[:], beta2[:], None,
                                    op0=Alu.mult)
            nalpha2 = sb.tile([16, 1], F32)
            nc.vector.tensor_scalar(nalpha2[:], alpha[:], -2.0, None,
                                    op0=Alu.mult)
            Rv = sb.tile([16, D], F32)
            nc.vector.scalar_tensor_tensor(Rv[:], statsS[:, 0:D], nalpha2[:],
                                           tmpd[:], op0=Alu.mult, op1=Alu.add)
            nc.vector.tensor_scalar(Rv[:], Rv[:], vmask[:], None, op0=Alu.mult)
            nc.tensor.transpose(pcoef[:, 0:16], Rv[:], idf)
            RTb = sb.tile([128, 32], BF16)
            nc.vector.tensor_copy(RTb[:, 0:16], pcoef[:, 0:16])
            nc.vector.tensor_copy(RTb[:, 16:32], pcoef[:, 0:16])

            ssd = sb.tile([16, 1], F32)
            nc.vector.tensor_tensor(ssd[:], gbP[:, D:D + 1], SqS[:],
                                    op=Alu.subtract)
            nc.vector.tensor_tensor(ssd[:], ssd[:], beta[:], op=Alu.mult)
            QM = sb.tile([16, 1], F32)
            nc.vector.scalar_tensor_tensor(QM[:], SqS[:], alpha[:], ssd[:],
                                           op0=Alu.mult, op1=Alu.subtract)
            nc.vector.tensor_scalar(QM[:], QM[:], MARGIN, vmask[:],
                                    op0=Alu.add, op1=Alu.mult)
            nc.tensor.matmul(pcoef[:, 32:33], ssel, QM[:], start=True,
                             stop=True)
            Qcg = sb.tile([128, 1], F32)
            nc.vector.tensor_copy(Qcg[:], pcoef[:, 32:33])

            # ---------------- loss chain A: f_i . R[c]  (h-major) --------
            for h in range(4):
                for g in range(NG):
                    nc.tensor.matmul(
                        lossP[32 * g:32 * g + 32, 512 * h:512 * (h + 1)],
                        RTb[:], fts[:, GI * g + 512 * h:GI * g + 512 * (h + 1)],
                        start=False, stop=(g == NG - 1),
                        tile_position=(0, 32 * g), skip_group_check=True)

            # ---------------- mask + relu + accumulate -------------------
            accq = sb.tile([128, 8], F32)
            nc.gpsimd.memset(accq[:], 0.0)
            nc.vector.tensor_tensor(accq[0:16, 4:5], cntS[:], vmask[:],
                                    op=Alu.mult)
            for h in range(4):
                t1 = sb.tile([128, 512], F32, tag="t1", bufs=2, name=f"t1_{h}")
                nc.vector.scalar_tensor_tensor(
                    t1[:], lossP[:, 512 * h:512 * (h + 1)], Qcg[:],
                    eohcs[:, 512 * h:512 * (h + 1)],
                    op0=Alu.add, op1=Alu.mult)
                rel = sb.tile([128, 512], F32, tag="rel", bufs=2,
                              name=f"rel_{h}")
                nc.vector.tensor_scalar(rel[:], t1[:], 0.0, None,
                                        op0=Alu.max, op1=Alu.add,
                                        accum_out=accq[:, h:h + 1])

            # ---------------- final reduction ----------------
            finP = ps.tile([1, 8], F32)
            nc.tensor.matmul(finP[:], cstbs[:], accq[:], start=True, stop=True)
            fin = sb.tile([1, 8], F32)
            nc.vector.tensor_copy(fin[:], finP[:])
            numS = sb.tile([1, 1], F32)
            nc.vector.tensor_reduce(numS[:], fin[:, 0:4], axis=AxX, op=Alu.add)
            den = sb.tile([1, 1], F32)
            nc.vector.tensor_scalar(den[:], fin[:, 4:5], 1.0, None, op0=Alu.max)
            nc.vector.reciprocal(den[:], den[:])
            resS = sb.tile([1, 1], F32)
            nc.vector.tensor_tensor(resS[:], numS[:], den[:], op=Alu.mult)
            nc.sync.dma_start(res, resS[:])

    nc.compile()
    _CACHE["nc"] = nc
    return nc


def _make_in_maps(features, labels):
    feats = np.ascontiguousarray(np.asarray(features, dtype=np.float32))
    lab = np.ascontiguousarray(np.asarray(labels)).astype(np.int64)
    fb = feats.astype(BF)                                   # (8192, 128)

    fr = np.zeros((128, T, W), BF)
    fr[:, :, 0:D] = fb.reshape(T, 128, D).transpose(1, 0, 2)
    fr[:, :, D] = 1.0
    ft = np.ascontiguousarray(fb.T)                         # (128, 8192)

    eoh = (lab[:, None] == np.arange(K)[None, :])           # (8192, 16)
    eohb = np.ascontiguousarray(
        eoh.reshape(T, 128, K).transpose(1, 0, 2).reshape(128, T * K)
    ).astype(BF)
    eohcg = np.zeros((128, GI), BF)
    for g in range(NG):
        eohcg[32 * g:32 * g + K, :] = eoh[g * GI:(g + 1) * GI, :].T

    csta = np.zeros((16, 272), np.float32)
    for g in range(NG):
        for q in range(K):
            csta[q, 32 * g + q] = 1.0
            csta[q, 32 * g + 16 + q] = 1.0
    csta[:, 128:144] = np.eye(16, dtype=np.float32)
    csta[:, 144:272] = 1.0
    cstb = np.ones((128, 1), np.float32)

    one = {
        "frd": np.ascontiguousarray(fr.reshape(128, T * W)),
        "ftd": ft,
        "eohb": eohb,
        "eohc": eohcg,
        "csta": csta,
        "cstb": cstb,
    }
    return [dict(one) for _ in range(NCORES)]


def kernel(features, labels):
    nc = _build()
    in_maps = _make_in_maps(features, labels)
    out = run_bass_kernel_spmd(nc, in_maps, core_ids=list(range(NCORES)))
    return np.float32(out.results[0]["res"][0, 0])
